# revision 15
# baseline (speedup 1.0000x reference)
"""Multi-head causal self-attention on 8 Trainium2 NeuronCores.

Tensor-parallel over heads: core i owns heads (2i, 2i+1). bf16 matmul
operands throughout (fp32 PSUM accumulation); tolerance is 2e-2.

Per core:
  phase 1: qT/kT/vT = (W_slice^T @ x^T) for its 2 heads; vT transposed on
           PE into [token, d] tiles (both heads + shared ones columns).
  phase 2: per (b, qj-block of 512 q, ki-chunk of 128 k), qj-outer:
           scoresT[k,q] for both heads packed as two row-tiled matmuls
           (head0 on PE rows 0-63, head1 on rows 64-127, concurrent);
           one Exp activation over the paired [128,1024] PSUM tile;
           causal-diagonal blocks narrowed to valid columns + [128,128]
           tril mask mul; PV accumulates [V_h | 1]^T @ attnT into
           po_h[65, 512] (row 64 = softmax denominator l).
           Per (b,h): copy po->araw (unnormalized + l), recip(l) on DVE,
           PE-broadcast 1/l, multiply into a_sb[128, 2048] (bf16).
  phase 3: per b: AllToAll shards a_sb by 256-token chunks, so core i
           ends up with [1024 features, 256 tokens] for tokens
           256i..256(i+1); local full W_proj^T @ A + bias -> outT chunk.
           A2A(b=0) overlaps b=1 attention.
Host reassembles the 8 token chunks per batch.
"""

import numpy as np

B, T, C, H = 2, 2048, 1024, 16
D = C // H            # 64
NCORES = 8
HL = H // NCORES      # 2 heads per core
NT = B * T            # 4096
NQ = T // 512         # 4 q-blocks of 512 per b
NK = T // 128         # 16 k-chunks of 128 per b
TCH = T // NCORES     # 256-token chunk per core per b (A2A shard)
SCALE = float(D) ** -0.5

_cache = {}


def _build(mode: str):
    """mode: 'causal' | 'none' (all-ones mask)."""
    import concourse.mybir as mybir
    import concourse.tile as tile
    from concourse import bacc

    f32 = mybir.dt.float32
    f32r = mybir.dt.float32r
    mdt = mybir.dt.bfloat16

    nc = bacc.Bacc("TRN2", target_bir_lowering=False, debug=False,
                   num_devices=NCORES)
    xT = nc.dram_tensor("xT", [C, NT], mdt, kind="ExternalInput").ap()
    wqkv = nc.dram_tensor("wqkv", [C, 3 * HL * D], mdt,
                          kind="ExternalInput").ap()
    wp = nc.dram_tensor("wp", [C, C], mdt, kind="ExternalInput").ap()
    bias = nc.dram_tensor("bias", [128, NCORES], f32,
                          kind="ExternalInput").ap()
    cmask = nc.dram_tensor("cmask", [128, 256], mdt,
                           kind="ExternalInput").ap()
    onesv = nc.dram_tensor("onesv", [128, 64 * B * NK], mdt,
                           kind="ExternalInput").ap()
    outT = nc.dram_tensor("outT", [C, B * TCH], f32,
                          kind="ExternalOutput").ap()

    causal = mode == "causal"
    Exp = mybir.ActivationFunctionType.Exp

    with tile.TileContext(nc) as tc, \
         nc.allow_low_precision(reason="bf16 matmul path, tol 2e-2"):
        with tc.tile_pool(name="persist", bufs=1) as persist, \
             tc.tile_pool(name="dram", bufs=1, space="DRAM") as dram:
            q_sb = persist.tile([128, NT], mdt)
            k_sb = persist.tile([128, NT], mdt)
            # V^T tiles per head h: cols 128h:128h+64 = V_h d-columns,
            # cols 128h+64:128h+128 = ones (so PV output rows 64:128 all
            # hold the softmax denominator l, replicated for cheap DVE
            # normalization straight out of PSUM).
            vboth = persist.tile([128, 256, B * NK], mdt)
            cm_sb = persist.tile([128, 256], mdt)
            wqkv_sb = persist.tile([128, 8, 3 * HL * D], mdt)
            wp_sb = persist.tile([128, 8, C], mdt)
            bias_sb = persist.tile([128, NCORES], f32)
            a2a_in0 = dram.tile([NCORES * 128, TCH], mdt)
            a2a_in1 = dram.tile([NCORES * 128, TCH], mdt)
            a2a_out0 = dram.tile([NCORES * 128, TCH], mdt)
            a2a_out1 = dram.tile([NCORES * 128, TCH], mdt)
            a2a_ins = [a2a_in0, a2a_in1]
            a2a_outs = [a2a_out0, a2a_out1]
            warm_in = dram.tile([NCORES, 16], mdt)
            warm_out = dram.tile([NCORES, 16], mdt)

            nc.sync.dma_start(out=wqkv_sb[:],
                              in_=wqkv.rearrange("(a p) n -> p a n", p=128))
            nc.gpsimd.dma_start(out=cm_sb[:], in_=cmask[:])
            nc.gpsimd.dma_start(out=bias_sb[:], in_=bias[:])
            nc.gpsimd.dma_start(
                out=vboth[:, 64:128, :],
                in_=onesv.rearrange("p (c j) -> p c j", c=64))
            nc.gpsimd.dma_start(
                out=vboth[:, 192:256, :],
                in_=onesv.rearrange("p (c j) -> p c j", c=64))
            nc.gpsimd.dma_start(out=wp_sb[:],
                                in_=wp.rearrange("(a p) n -> p a n", p=128))
            ident = cm_sb[:, 128:256]

            # PSUM layout (8 banks):
            #   mm1 (2 banks): phase-1 qkv ps + v-transposes + norm rb +
            #                  proj pr, all via shared slot group
            #   sc  (4 banks): paired score tiles [128,1024]
            #   po  (2 banks): po_h0 / po_h1 accumulators
            with tc.tile_pool(name="mm1", bufs=2, space="PSUM") as mm1, \
                 tc.tile_pool(name="sc_psum", bufs=2, space="PSUM") as scp, \
                 tc.tile_pool(name="po_psum", bufs=1, space="PSUM") as pop, \
                 tc.tile_pool(name="xn_pool", bufs=2) as xp, \
                 tc.tile_pool(name="vtmp_pool", bufs=2) as vpool, \
                 tc.tile_pool(name="at_pool", bufs=4) as apool, \
                 tc.tile_pool(name="rb_pool", bufs=2) as rbp, \
                 tc.tile_pool(name="a_pool", bufs=2) as ap_pool, \
                 tc.tile_pool(name="agt_pool", bufs=2) as agp, \
                 tc.tile_pool(name="out_pool", bufs=3) as outp:

                def qkv_group(ng):
                    """QKV projection for token blocks ng*2048..(+2048)."""
                    xn = xp.tile([128, 8, 2048], mdt, tag="xn", name="xn")
                    for nl_ in range(4):
                        for kc in range(8):
                            nc.sync.dma_start(
                                out=xn[:, kc, nl_ * 512:(nl_ + 1) * 512],
                                in_=xT[kc * 128:(kc + 1) * 128,
                                       ng * 2048 + nl_ * 512:
                                       ng * 2048 + (nl_ + 1) * 512])
                    for nl in range(4):
                        n = ng * 4 + nl
                        tok = slice(n * 512, (n + 1) * 512)
                        for m in range(3):  # 0: q, 1: k, 2: v
                            ps = mm1.tile([128, 512], f32, tag="ps",
                                          name="ps")
                            for kc in range(8):
                                nc.tensor.matmul(
                                    ps[:],
                                    wqkv_sb[:, kc, m * 128:(m + 1) * 128],
                                    xn[:, kc, nl * 512:(nl + 1) * 512],
                                    start=(kc == 0), stop=(kc == 7))
                            if m == 0:
                                nc.vector.tensor_copy(q_sb[:, tok], ps[:])
                            elif m == 1:
                                nc.vector.tensor_copy(k_sb[:, tok], ps[:])
                            else:
                                vtmp = vpool.tile([128, 512], mdt,
                                                  tag="vtmp", name="vtmp")
                                nc.vector.tensor_copy(vtmp[:], ps[:])
                                bb = n // NQ
                                for s in range(4):
                                    j = bb * NK + (n % NQ) * 4 + s
                                    pt = mm1.tile([128, 128], mdt, tag="ps",
                                                  name="pt")
                                    nc.tensor.transpose(
                                        pt[:],
                                        vtmp[:, s * 128:(s + 1) * 128],
                                        ident)
                                    nc.vector.tensor_copy(
                                        vboth[:, 0:64, j], pt[:, 0:64])
                                    nc.vector.tensor_copy(
                                        vboth[:, 128:192, j],
                                        pt[:, 64:128])

                def attn_loops(b, a_sb, qjs):
                    """Score/exp/PV loops + normalization for batch b."""
                    for qj in qjs:
                        last_ki = 4 * qj + 3 if causal else NK - 1
                        po0 = pop.tile([128, 512], f32, tag="po0",
                                       name="po0")
                        po1 = pop.tile([128, 512], f32, tag="po1",
                                       name="po1")
                        pos = [po0, po1]
                        for ki in range(last_ki + 1):
                            diag = causal and ki >= 4 * qj
                            st = (ki - 4 * qj) * 128 if diag else 0
                            kc_ = slice(b * T + ki * 128,
                                        b * T + (ki + 1) * 128)
                            qc = slice(b * T + qj * 512 + st,
                                       b * T + (qj + 1) * 512)
                            sc = scp.tile([128, 1024], f32, tag="sc",
                                          name="sc")
                            nc.tensor.matmul(
                                sc[:, st:512], k_sb[0:64, kc_],
                                q_sb[0:64, qc], start=True, stop=True)
                            nc.tensor.matmul(
                                sc[:, 512 + st:1024], k_sb[64:128, kc_],
                                q_sb[64:128, qc], start=True, stop=True)
                            at = apool.tile([128, 1024], mdt, tag="at",
                                            name="at")
                            if diag:
                                at3 = at[:].rearrange(
                                    "p (c t) -> p c t", c=2)[:, :, st:512]
                                sc3 = sc[:].rearrange(
                                    "p (c t) -> p c t", c=2)[:, :, st:512]
                                nc.scalar.activation(at3, sc3, Exp,
                                                     scale=SCALE)
                                nc.vector.tensor_mul(
                                    at[:, st:st + 128], at[:, st:st + 128],
                                    cm_sb[:, 0:128])
                                nc.vector.tensor_mul(
                                    at[:, 512 + st:512 + st + 128],
                                    at[:, 512 + st:512 + st + 128],
                                    cm_sb[:, 0:128])
                            else:
                                nc.scalar.activation(at[:], sc[:], Exp,
                                                     scale=SCALE)
                            vj = b * NK + ki
                            for h in range(2):
                                nc.tensor.matmul(
                                    pos[h][:, st:512],
                                    vboth[:, 128 * h:128 * h + 128, vj],
                                    at[:, 512 * h + st:512 * h + 512],
                                    start=(ki == 0), stop=(ki == last_ki))
                        # normalize: po rows 64:128 are l replicated
                        for h in range(2):
                            lrep = rbp.tile([64, 512], f32, tag="lrep",
                                            name="lrep")
                            nc.vector.tensor_copy(lrep[:],
                                                  pos[h][64:128, :])
                            rb = rbp.tile([64, 512], f32, tag="rb",
                                          name="rb")
                            nc.vector.reciprocal_approx_fast(rb[:],
                                                             lrep[:])
                            nc.vector.tensor_mul(
                                a_sb[64 * h:64 * h + 64,
                                     qj * 512:(qj + 1) * 512],
                                pos[h][0:64, :], rb[:])

                def stage_b(b, a_sb):
                    nc.sync.dma_start(
                        out=a2a_ins[b].rearrange("(c p) t -> p c t", p=128),
                        in_=a_sb[:].rearrange("p (c t) -> p c t",
                                              c=NCORES))

                def a2a_b(b):
                    nc.gpsimd.collective_compute(
                        "AllToAll", mybir.AluOpType.bypass,
                        replica_groups=[list(range(NCORES))],
                        ins=[a2a_ins[b].opt()], outs=[a2a_outs[b].opt()])

                def proj_b(b):
                    """Local projection for batch b's token chunk."""
                    agt = agp.tile([128, 8, TCH], mdt, tag="agt",
                                   name="agt")
                    nc.sync.dma_start(
                        out=agt[:],
                        in_=a2a_outs[b].rearrange("(c p) t -> p c t",
                                                  p=128))
                    for o in range(8):
                        pr = mm1.tile([128, TCH], f32, tag="ps", name="pr")
                        for kc in range(8):
                            nc.tensor.matmul(
                                pr[:], wp_sb[:, kc, o * 128:(o + 1) * 128],
                                agt[:, kc, :],
                                start=(kc == 0), stop=(kc == 7))
                        ot = outp.tile([128, TCH], f32, tag="ot", name="ot")
                        nc.vector.tensor_scalar_add(ot[:], pr[:],
                                                    bias_sb[:, o:o + 1])
                        nc.sync.dma_start(
                            out=outT[o * 128:(o + 1) * 128,
                                     b * TCH:(b + 1) * TCH],
                            in_=ot[:])

                # tiny warmup collective to absorb first-op CC latency
                nc.gpsimd.collective_compute(
                    "AllToAll", mybir.AluOpType.bypass,
                    replica_groups=[list(range(NCORES))],
                    ins=[warm_in.opt()], outs=[warm_out.opt()])
                a_sb0 = ap_pool.tile([128, T], mdt, tag="a_sb",
                                     name="a_sb0")
                a_sb1 = ap_pool.tile([128, T], mdt, tag="a_sb",
                                     name="a_sb1")
                qkv_group(0)
                attn_loops(0, a_sb0, [0, 1])
                qkv_group(1)
                attn_loops(0, a_sb0, [2, 3])
                stage_b(0, a_sb0)
                a2a_b(0)
                attn_loops(1, a_sb1, range(NQ))
                stage_b(1, a_sb1)
                a2a_b(1)
                # proj(b0) PE work hides A2A(b1) flight time
                proj_b(0)
                proj_b(1)

    nc.compile()
    return nc


def _get_program(mode: str):
    if mode not in _cache:
        _cache[mode] = _build(mode)
    return _cache[mode]


def kernel(**inputs):
    import ml_dtypes
    from concourse.bass_utils import run_bass_kernel_spmd

    bf16 = ml_dtypes.bfloat16

    x = np.asarray(inputs["x"], dtype=np.float32)
    mask = np.asarray(inputs["causal_mask"])
    Wqkv = np.asarray(inputs["W_qkv"], dtype=np.float32)
    Wp = np.asarray(inputs["W_proj"], dtype=np.float32)
    bp = np.asarray(inputs["b_proj"], dtype=np.float32)

    m2 = mask.reshape(T, T)
    if np.all(m2 != 0):
        mode = "none"
    else:
        tril = np.tril(np.ones((T, T), dtype=m2.dtype))
        if np.array_equal(m2, tril):
            mode = "causal"
        else:
            raise NotImplementedError("general mask not supported")

    nc = _get_program(mode)

    xT = np.ascontiguousarray(x.reshape(NT, C).T).astype(bf16)

    # [128,256]: triu mask (k<=q within a diagonal 128-block) | identity
    p = np.arange(128)[:, None]
    f = np.arange(128)[None, :]
    cm = np.concatenate(
        [(p <= f).astype(np.float32), np.eye(128, dtype=np.float32)],
        axis=1).astype(bf16)

    Wq = Wqkv[:, 0 * C:1 * C]
    Wk = Wqkv[:, 1 * C:2 * C]
    Wv = Wqkv[:, 2 * C:3 * C]
    wp_bf = np.ascontiguousarray(Wp).astype(bf16)
    bias_h = np.ascontiguousarray(bp.reshape(NCORES, 128).T)

    in_maps = []
    for i in range(NCORES):
        hcols = slice(2 * i * D, (2 * i + 2) * D)  # this core's 2 heads
        wqkv_i = np.concatenate(
            [Wq[:, hcols], Wk[:, hcols], Wv[:, hcols]], axis=1)
        in_maps.append({
            "xT": xT,
            "wqkv": np.ascontiguousarray(wqkv_i).astype(bf16),
            "wp": wp_bf,
            "bias": bias_h,
            "cmask": cm,
            "onesv": np.ones((128, 64 * B * NK), dtype=bf16),
        })

    res = run_bass_kernel_spmd(nc, in_maps, list(range(NCORES)))

    out = np.empty((B, T, C), dtype=np.float32)
    for i in range(NCORES):
        oT = res.results[i]["outT"]  # [C, B*TCH] f32
        for b in range(B):
            out[b, i * TCH:(i + 1) * TCH, :] = \
                oT[:, b * TCH:(b + 1) * TCH].T
    return out


# revision 16
# speedup vs baseline: 1.0185x; 1.0185x over previous
"""Multi-head causal self-attention on 8 Trainium2 NeuronCores.

Tensor-parallel over heads: core i owns heads (2i, 2i+1). bf16 matmul
operands throughout (fp32 PSUM accumulation); harness tolerance 2e-2.

Per core:
  phase 1: qT/kT/vT = (W_slice^T @ x^T) for its 2 heads; vT transposed
           on the PE into [token, d] tiles. Emission interleaves the
           second token-group's QKV with batch-0 attention to keep the
           PE dense (HAM stays un-throttled).
  phase 2: per (b, qj of 512 q, ki of 128 k), qj-outer: scoresT[k,q]
           for both heads as two row-tiled matmuls (head0 on PE rows
           0-63, head1 on rows 64-127, running concurrently); one Exp
           activation over the paired [128,1024] 2-bank PSUM tile (3D
           strided AP fuses the two heads on causal-diagonal blocks,
           which are narrowed to their valid columns + a [128,128]
           tril mask multiply). PV accumulates [V_h | ones*64]^T @
           attnT into po_h[128, 512]: rows 64:128 all hold the softmax
           denominator l, so normalization is copy(l)+recip+mul on DVE
           straight out of PSUM - no broadcast matmuls, inline per qj.
  phase 3: per b: AllToAll shards a_sb by 256-token chunks so core i
           ends up with [1024 features, 256 tokens] for tokens
           256i..256(i+1); local full W_proj^T @ A + bias -> outT.
           A2A(b0) flies under b1's attention; A2A(b1) under proj(b0).
           A tiny warmup AllToAll at kernel start absorbs the ~11us
           first-collective latency.
Host reassembles the 8 token chunks per batch.
"""

import numpy as np

B, T, C, H = 2, 2048, 1024, 16
D = C // H            # 64
NCORES = 8
HL = H // NCORES      # 2 heads per core
NT = B * T            # 4096
NQ = T // 512         # 4 q-blocks of 512 per b
NK = T // 128         # 16 k-chunks of 128 per b
TCH = T // NCORES     # 256-token chunk per core per b (A2A shard)
SCALE = float(D) ** -0.5

_cache = {}


def _build(mode: str):
    """mode: 'causal' | 'none' (all-ones mask)."""
    import concourse.mybir as mybir
    import concourse.tile as tile
    from concourse import bacc

    f32 = mybir.dt.float32
    f32r = mybir.dt.float32r
    mdt = mybir.dt.bfloat16

    nc = bacc.Bacc("TRN2", target_bir_lowering=False, debug=False,
                   num_devices=NCORES)
    xT = nc.dram_tensor("xT", [C, NT], mdt, kind="ExternalInput").ap()
    wqkv = nc.dram_tensor("wqkv", [C, 3 * HL * D], mdt,
                          kind="ExternalInput").ap()
    wp = nc.dram_tensor("wp", [C, C], mdt, kind="ExternalInput").ap()
    bias = nc.dram_tensor("bias", [128, NCORES], f32,
                          kind="ExternalInput").ap()
    cmask = nc.dram_tensor("cmask", [128, 256], mdt,
                           kind="ExternalInput").ap()
    onesv = nc.dram_tensor("onesv", [128, 64 * B * NK], mdt,
                           kind="ExternalInput").ap()
    outT = nc.dram_tensor("outT", [C, B * TCH], f32,
                          kind="ExternalOutput").ap()

    causal = mode == "causal"
    Exp = mybir.ActivationFunctionType.Exp

    with tile.TileContext(nc) as tc, \
         nc.allow_low_precision(reason="bf16 matmul path, tol 2e-2"):
        with tc.tile_pool(name="persist", bufs=1) as persist, \
             tc.tile_pool(name="dram", bufs=1, space="DRAM") as dram:
            q_sb = persist.tile([128, NT], mdt)
            k_sb = persist.tile([128, NT], mdt)
            # V^T tiles per head h: cols 128h:128h+64 = V_h d-columns,
            # cols 128h+64:128h+128 = ones (so PV output rows 64:128 all
            # hold the softmax denominator l, replicated for cheap DVE
            # normalization straight out of PSUM).
            vboth = persist.tile([128, 256, B * NK], mdt)
            cm_sb = persist.tile([128, 256], mdt)
            wqkv_sb = persist.tile([128, 8, 3 * HL * D], mdt)
            wp_sb = persist.tile([128, 8, C], mdt)
            bias_sb = persist.tile([128, NCORES], f32)
            a2a_in0 = dram.tile([NCORES * 128, TCH], mdt)
            a2a_in1 = dram.tile([NCORES * 128, TCH], mdt)
            a2a_out0 = dram.tile([NCORES * 128, TCH], mdt)
            a2a_out1 = dram.tile([NCORES * 128, TCH], mdt)
            a2a_ins = [a2a_in0, a2a_in1]
            a2a_outs = [a2a_out0, a2a_out1]
            warm_in = dram.tile([NCORES, 16], mdt)
            warm_out = dram.tile([NCORES, 16], mdt)

            nc.sync.dma_start(out=wqkv_sb[:],
                              in_=wqkv.rearrange("(a p) n -> p a n", p=128))
            nc.gpsimd.dma_start(out=cm_sb[:], in_=cmask[:])
            nc.gpsimd.dma_start(out=bias_sb[:], in_=bias[:])
            nc.gpsimd.dma_start(
                out=vboth[:, 64:128, :],
                in_=onesv.rearrange("p (c j) -> p c j", c=64))
            nc.gpsimd.dma_start(
                out=vboth[:, 192:256, :],
                in_=onesv.rearrange("p (c j) -> p c j", c=64))
            nc.gpsimd.dma_start(out=wp_sb[:],
                                in_=wp.rearrange("(a p) n -> p a n", p=128))
            ident = cm_sb[:, 128:256]

            # PSUM layout (8 banks):
            #   mm1 (2 banks): phase-1 qkv ps + v-transposes + norm rb +
            #                  proj pr, all via shared slot group
            #   sc  (4 banks): paired score tiles [128,1024]
            #   po  (2 banks): po_h0 / po_h1 accumulators
            with tc.tile_pool(name="mm1", bufs=2, space="PSUM") as mm1, \
                 tc.tile_pool(name="sc_psum", bufs=2, space="PSUM") as scp, \
                 tc.tile_pool(name="po_psum", bufs=1, space="PSUM") as pop, \
                 tc.tile_pool(name="xn_pool", bufs=2) as xp, \
                 tc.tile_pool(name="vtmp_pool", bufs=2) as vpool, \
                 tc.tile_pool(name="at_pool", bufs=4) as apool, \
                 tc.tile_pool(name="rb_pool", bufs=2) as rbp, \
                 tc.tile_pool(name="a_pool", bufs=2) as ap_pool, \
                 tc.tile_pool(name="agt_pool", bufs=2) as agp, \
                 tc.tile_pool(name="out_pool", bufs=3) as outp:

                def qkv_group(ng):
                    """QKV projection for token blocks ng*2048..(+2048)."""
                    xn = xp.tile([128, 8, 2048], mdt, tag="xn", name="xn")
                    for nl_ in range(4):
                        for kc in range(8):
                            nc.sync.dma_start(
                                out=xn[:, kc, nl_ * 512:(nl_ + 1) * 512],
                                in_=xT[kc * 128:(kc + 1) * 128,
                                       ng * 2048 + nl_ * 512:
                                       ng * 2048 + (nl_ + 1) * 512])
                    for nl in range(4):
                        n = ng * 4 + nl
                        tok = slice(n * 512, (n + 1) * 512)
                        for m in range(3):  # 0: q, 1: k, 2: v
                            ps = mm1.tile([128, 512], f32, tag="ps",
                                          name="ps")
                            for kc in range(8):
                                nc.tensor.matmul(
                                    ps[:],
                                    wqkv_sb[:, kc, m * 128:(m + 1) * 128],
                                    xn[:, kc, nl * 512:(nl + 1) * 512],
                                    start=(kc == 0), stop=(kc == 7))
                            if m == 0:
                                nc.vector.tensor_copy(q_sb[:, tok], ps[:])
                            elif m == 1:
                                nc.vector.tensor_copy(k_sb[:, tok], ps[:])
                            else:
                                vtmp = vpool.tile([128, 512], mdt,
                                                  tag="vtmp", name="vtmp")
                                nc.vector.tensor_copy(vtmp[:], ps[:])
                                bb = n // NQ
                                for s in range(4):
                                    j = bb * NK + (n % NQ) * 4 + s
                                    pt = mm1.tile([128, 128], mdt, tag="ps",
                                                  name="pt")
                                    nc.tensor.transpose(
                                        pt[:],
                                        vtmp[:, s * 128:(s + 1) * 128],
                                        ident)
                                    nc.vector.tensor_copy(
                                        vboth[:, 0:64, j], pt[:, 0:64])
                                    nc.vector.tensor_copy(
                                        vboth[:, 128:192, j],
                                        pt[:, 64:128])

                def attn_loops(b, a_sb, qjs):
                    """Score/exp/PV loops + normalization for batch b."""
                    for qj in qjs:
                        last_ki = 4 * qj + 3 if causal else NK - 1
                        po0 = pop.tile([128, 512], f32, tag="po0",
                                       name="po0")
                        po1 = pop.tile([128, 512], f32, tag="po1",
                                       name="po1")
                        pos = [po0, po1]
                        for ki in range(last_ki + 1):
                            diag = causal and ki >= 4 * qj
                            st = (ki - 4 * qj) * 128 if diag else 0
                            kc_ = slice(b * T + ki * 128,
                                        b * T + (ki + 1) * 128)
                            qc = slice(b * T + qj * 512 + st,
                                       b * T + (qj + 1) * 512)
                            sc = scp.tile([128, 1024], f32, tag="sc",
                                          name="sc")
                            nc.tensor.matmul(
                                sc[:, st:512], k_sb[0:64, kc_],
                                q_sb[0:64, qc], start=True, stop=True)
                            nc.tensor.matmul(
                                sc[:, 512 + st:1024], k_sb[64:128, kc_],
                                q_sb[64:128, qc], start=True, stop=True)
                            at = apool.tile([128, 1024], mdt, tag="at",
                                            name="at")
                            if diag:
                                at3 = at[:].rearrange(
                                    "p (c t) -> p c t", c=2)[:, :, st:512]
                                sc3 = sc[:].rearrange(
                                    "p (c t) -> p c t", c=2)[:, :, st:512]
                                nc.scalar.activation(at3, sc3, Exp,
                                                     scale=SCALE)
                                nc.vector.tensor_mul(
                                    at[:, st:st + 128], at[:, st:st + 128],
                                    cm_sb[:, 0:128])
                                nc.vector.tensor_mul(
                                    at[:, 512 + st:512 + st + 128],
                                    at[:, 512 + st:512 + st + 128],
                                    cm_sb[:, 0:128])
                            else:
                                nc.scalar.activation(at[:], sc[:], Exp,
                                                     scale=SCALE)
                            vj = b * NK + ki
                            for h in range(2):
                                nc.tensor.matmul(
                                    pos[h][:, st:512],
                                    vboth[:, 128 * h:128 * h + 128, vj],
                                    at[:, 512 * h + st:512 * h + 512],
                                    start=(ki == 0), stop=(ki == last_ki))
                        # normalize: po rows 64:128 are l replicated
                        for h in range(2):
                            lrep = rbp.tile([64, 512], f32, tag="lrep",
                                            name="lrep")
                            nc.vector.tensor_copy(lrep[:],
                                                  pos[h][64:128, :])
                            rb = rbp.tile([64, 512], f32, tag="rb",
                                          name="rb")
                            nc.vector.reciprocal_approx_fast(rb[:],
                                                             lrep[:])
                            nc.vector.tensor_mul(
                                a_sb[64 * h:64 * h + 64,
                                     qj * 512:(qj + 1) * 512],
                                pos[h][0:64, :], rb[:])

                def stage_b(b, a_sb):
                    nc.sync.dma_start(
                        out=a2a_ins[b].rearrange("(c p) t -> p c t", p=128),
                        in_=a_sb[:].rearrange("p (c t) -> p c t",
                                              c=NCORES))

                def a2a_b(b):
                    nc.gpsimd.collective_compute(
                        "AllToAll", mybir.AluOpType.bypass,
                        replica_groups=[list(range(NCORES))],
                        ins=[a2a_ins[b].opt()], outs=[a2a_outs[b].opt()])

                def proj_b(b):
                    """Local projection for batch b's token chunk."""
                    agt = agp.tile([128, 8, TCH], mdt, tag="agt",
                                   name="agt")
                    nc.sync.dma_start(
                        out=agt[:],
                        in_=a2a_outs[b].rearrange("(c p) t -> p c t",
                                                  p=128))
                    for o in range(8):
                        pr = mm1.tile([128, TCH], f32, tag="ps", name="pr")
                        for kc in range(8):
                            nc.tensor.matmul(
                                pr[:], wp_sb[:, kc, o * 128:(o + 1) * 128],
                                agt[:, kc, :],
                                start=(kc == 0), stop=(kc == 7))
                        ot = outp.tile([128, TCH], f32, tag="ot", name="ot")
                        nc.vector.tensor_scalar_add(ot[:], pr[:],
                                                    bias_sb[:, o:o + 1])
                        nc.sync.dma_start(
                            out=outT[o * 128:(o + 1) * 128,
                                     b * TCH:(b + 1) * TCH],
                            in_=ot[:])

                # tiny warmup collective to absorb first-op CC latency
                nc.gpsimd.collective_compute(
                    "AllToAll", mybir.AluOpType.bypass,
                    replica_groups=[list(range(NCORES))],
                    ins=[warm_in.opt()], outs=[warm_out.opt()])
                a_sb0 = ap_pool.tile([128, T], mdt, tag="a_sb",
                                     name="a_sb0")
                a_sb1 = ap_pool.tile([128, T], mdt, tag="a_sb",
                                     name="a_sb1")
                qkv_group(0)
                attn_loops(0, a_sb0, [0, 1])
                qkv_group(1)
                attn_loops(0, a_sb0, [2, 3])
                stage_b(0, a_sb0)
                a2a_b(0)
                attn_loops(1, a_sb1, range(NQ))
                stage_b(1, a_sb1)
                a2a_b(1)
                # proj(b0) PE work hides A2A(b1) flight time
                proj_b(0)
                proj_b(1)

    nc.compile()
    return nc


def _get_program(mode: str):
    if mode not in _cache:
        _cache[mode] = _build(mode)
    return _cache[mode]


def kernel(**inputs):
    import ml_dtypes
    from concourse.bass_utils import run_bass_kernel_spmd

    bf16 = ml_dtypes.bfloat16

    x = np.asarray(inputs["x"], dtype=np.float32)
    mask = np.asarray(inputs["causal_mask"])
    Wqkv = np.asarray(inputs["W_qkv"], dtype=np.float32)
    Wp = np.asarray(inputs["W_proj"], dtype=np.float32)
    bp = np.asarray(inputs["b_proj"], dtype=np.float32)

    m2 = mask.reshape(T, T)
    if np.all(m2 != 0):
        mode = "none"
    else:
        tril = np.tril(np.ones((T, T), dtype=m2.dtype))
        if np.array_equal(m2, tril):
            mode = "causal"
        else:
            raise NotImplementedError("general mask not supported")

    nc = _get_program(mode)

    xT = np.ascontiguousarray(x.reshape(NT, C).T).astype(bf16)

    # [128,256]: triu mask (k<=q within a diagonal 128-block) | identity
    p = np.arange(128)[:, None]
    f = np.arange(128)[None, :]
    cm = np.concatenate(
        [(p <= f).astype(np.float32), np.eye(128, dtype=np.float32)],
        axis=1).astype(bf16)

    Wq = Wqkv[:, 0 * C:1 * C]
    Wk = Wqkv[:, 1 * C:2 * C]
    Wv = Wqkv[:, 2 * C:3 * C]
    wp_bf = np.ascontiguousarray(Wp).astype(bf16)
    bias_h = np.ascontiguousarray(bp.reshape(NCORES, 128).T)

    in_maps = []
    for i in range(NCORES):
        hcols = slice(2 * i * D, (2 * i + 2) * D)  # this core's 2 heads
        wqkv_i = np.concatenate(
            [Wq[:, hcols], Wk[:, hcols], Wv[:, hcols]], axis=1)
        in_maps.append({
            "xT": xT,
            "wqkv": np.ascontiguousarray(wqkv_i).astype(bf16),
            "wp": wp_bf,
            "bias": bias_h,
            "cmask": cm,
            "onesv": np.ones((128, 64 * B * NK), dtype=bf16),
        })

    res = run_bass_kernel_spmd(nc, in_maps, list(range(NCORES)))

    out = np.empty((B, T, C), dtype=np.float32)
    for i in range(NCORES):
        oT = res.results[i]["outT"]  # [C, B*TCH] f32
        for b in range(B):
            out[b, i * TCH:(i + 1) * TCH, :] = \
                oT[:, b * TCH:(b + 1) * TCH].T
    return out
